# revision 22
# baseline (speedup 1.0000x reference)
"""Causal multi-head self-attention on 8 Trainium2 NeuronCores.

Sharding: core = (batch b, head-group g).  B=4 batches x 2 groups of 8 heads
= 8 cores.  Each core computes Q/K/V projections for its 8 heads, causal
attention, and a partial output projection (row-shard of WO); the host sums
the two partials per batch (the tensor-parallel all-reduce, done at gather).

v6 (bf16): the attention phase is paced by ScalarE exp (~312 cyc fixed
overhead per ACTIVATE + 1 elem/cycle/lane), so everything else is arranged
to keep the exp stream dense:
  - score tiles for a head PAIR live in one 2-bank PSUM tile [128, 2, 512];
    each k-chunk gets ONE exp covering both heads;
  - the causal stripe mask is a 0/1 multiply on the et tile by the
    otherwise-idle GpSimd engine, off both DVE and the scores->exp path;
  - diagonal-chunk scores/exps are trimmed to the allowed columns [cs:];
  - the scores/attnV software pipeline (attnV lags scores by one chunk) runs
    CONTINUOUSLY across pair and q-block boundaries;
  - the previous q-block's normalization + output projection is sliced into
    small pieces pumped one-per-chunk between attnV groups, filling the PE
    slack under the exp pacing instead of stalling ACT;
  - each 512-wide half of the output projection is a separate 4-matmul
    same-bank accumulation chain (interleaving the halves costs a PE
    pipeline drain per matmul: 760 vs 455 ns measured);
  - ScalarE does nothing but exp in the attention phase.

Stage 1 V runs first per s-block so compute starts as soon as wv + the first
xT chunk land.
"""

import os
import numpy as np
import ml_dtypes

B, S, D = 4, 2048, 1024
H_TOTAL, DK = 16, 64
G = 2          # head groups (cores per batch)
HG = 8         # heads per core
DG = 512       # head dims per core
CO = 8         # contraction chunks of 128 over D
SBLK = 4       # 512-wide s blocks
QB = 4         # 512-wide q blocks
NEG = -1e9

_BUILD_CACHE = {}


def _build():
    if "nc" in _BUILD_CACHE:
        return _BUILD_CACHE["nc"]

    import concourse.bacc as bacc
    import concourse.mybir as mybir
    import concourse.tile as tile
    from concourse.tile_rust import add_dep_helper

    f32 = mybir.dt.float32
    f32r = mybir.dt.float32r
    bf16 = mybir.dt.bfloat16
    AF = mybir.ActivationFunctionType
    MULT = mybir.AluOpType.mult

    nc = bacc.Bacc("TRN2", target_bir_lowering=False)
    xT_d = nc.dram_tensor("xT", [D, S], bf16, kind="ExternalInput")
    wq_d = nc.dram_tensor("wqT", [D, DG], bf16, kind="ExternalInput")
    wk_d = nc.dram_tensor("wkT", [D, DG], bf16, kind="ExternalInput")
    wv_d = nc.dram_tensor("wvT", [D, DG], bf16, kind="ExternalInput")
    wo_d = nc.dram_tensor("woT", [DG, D], bf16, kind="ExternalInput")
    mask_d = nc.dram_tensor("mask", [128, 256], bf16, kind="ExternalInput")
    ones_d = nc.dram_tensor("onesb", [128, 128], bf16, kind="ExternalInput")
    ones32b_d = nc.dram_tensor("ones32", [128, 64], bf16, kind="ExternalInput")
    out_d = nc.dram_tensor("out", [S, D], bf16, kind="ExternalOutput")

    with tile.TileContext(nc) as tc:
        with tc.tile_pool(name="persist", bufs=1) as pp:
            QT = pp.tile([128, 4, S], bf16, tag="QT")
            KT = pp.tile([128, 4, S], bf16, tag="KT")
            V = pp.tile([128, 16, HG, DK + 1], bf16, tag="V")
            AT = pp.tile([128, 4, S], bf16, tag="AT")
            wo = pp.tile([128, 4, D], bf16, tag="wo")
            maskb = pp.tile([128, 2, 128], bf16, tag="maskb")
            onesb = pp.tile([128, 128], bf16, tag="onesb")
            ones_r = pp.tile([128, 64], bf16, tag="ones_r")
            nc.sync.dma_start(maskb[:], mask_d[:, :].rearrange("p (h c) -> p h c", h=2))
            nc.sync.dma_start(onesb[:], ones_d[:, :])
            nc.sync.dma_start(ones_r[:], ones32b_d[:, :])
            # ones column of V (bf16 broadcast copy)
            nc.vector.tensor_copy(
                V[:, :, :, DK : DK + 1],
                onesb[:, 0:128].rearrange("p (so h) -> p so h", so=16)[:, :, :, None],
            )

            # ---------------- stage 1: projections ----------------
            with (
                tc.tile_pool(name="stage1", bufs=1) as s1p,
                tc.tile_pool(name="ps1", bufs=1, space="PSUM") as ps1,
            ):
                wv = s1p.tile([128, CO, DG], bf16, tag="wv")
                wq = s1p.tile([128, CO, DG], bf16, tag="wq")
                wk = s1p.tile([128, CO, DG], bf16, tag="wk")
                nc.sync.dma_start(
                    wv[:, 0:4, :],
                    wv_d[0:512, :].rearrange("(co ci) d -> ci co d", ci=128),
                )
                nc.sync.dma_start(
                    wv[:, 4:8, :],
                    wv_d[512:1024, :].rearrange("(co ci) d -> ci co d", ci=128),
                )
                first = True
                for sb in range(SBLK):
                    xt = s1p.tile([128, CO, 512], bf16, tag="xt", bufs=2)
                    ssl = slice(sb * 512, (sb + 1) * 512)
                    if first:
                        nc.sync.dma_start(
                            xt[:, 0:4, :],
                            xT_d[0:512, ssl].rearrange("(co ci) s -> ci co s", ci=128),
                        )
                        nc.sync.dma_start(
                            xt[:, 4:8, :],
                            xT_d[512:1024, ssl].rearrange("(co ci) s -> ci co s", ci=128),
                        )
                    else:
                        nc.sync.dma_start(
                            xt, xT_d[:, ssl].rearrange("(co ci) s -> ci co s", ci=128)
                        )
                    if first:
                        # queue the remaining weight loads behind the first
                        # x chunk so V(sb0) can start after ~2MB of DMA
                        nc.sync.dma_start(
                            wq, wq_d[:, :].rearrange("(co ci) d -> ci co d", ci=128)
                        )
                        nc.sync.dma_start(
                            wk, wk_d[:, :].rearrange("(co ci) d -> ci co d", ci=128)
                        )
                        nc.sync.dma_start(
                            wo, wo_d[:, :].rearrange("(io ip) j -> ip io j", ip=128)
                        )
                        first = False
                    for so in range(4):
                        sc = sb * 4 + so
                        pv = ps1.tile([128, 512], f32, tag="proj", bufs=4, name=f"pv{sc}")
                        for co in range(CO):
                            nc.tensor.matmul(
                                pv, xt[:, co, so * 128 : (so + 1) * 128], wv[:, co, :],
                                start=(co == 0), stop=(co == CO - 1),
                            )
                        nc.vector.tensor_copy(
                            V[:, sc, :, 0:DK],
                            pv[:].rearrange("p (h d) -> p h d", h=HG),
                        )
                    for do in range(4):
                        dsl = slice(do * 128, (do + 1) * 128)
                        pq = ps1.tile([128, 512], f32, tag="proj", bufs=4, name=f"pq{sb}{do}")
                        for co in range(CO):
                            nc.tensor.matmul(
                                pq, wq[:, co, dsl], xt[:, co, :],
                                start=(co == 0), stop=(co == CO - 1),
                            )
                        nc.vector.tensor_copy(QT[:, do, ssl], pq[:])
                        pk = ps1.tile([128, 512], f32, tag="proj", bufs=4, name=f"pk{sb}{do}")
                        for co in range(CO):
                            nc.tensor.matmul(
                                pk, wk[:, co, dsl], xt[:, co, :],
                                start=(co == 0), stop=(co == CO - 1),
                            )
                        nc.vector.tensor_copy(KT[:, do, ssl], pk[:])

            # ---------------- stages 2+3: attention + output ----------------
            with (
                tc.tile_pool(name="stage2", bufs=1) as s2p,
                tc.tile_pool(name="ps2", bufs=1, space="PSUM") as ps2,
            ):
                pe_prev = [None]  # last instr of the previous PE group

                def pe_group(insts):
                    if not insts:
                        return
                    if pe_prev[0] is not None:
                        add_dep_helper(
                            insts[0].ins, pe_prev[0].ins, sync=False,
                            reason="pe group order",
                        )
                    for a, b in zip(insts[1:], insts):
                        add_dep_helper(a.ins, b.ins, sync=False, reason="pe chain")
                    pe_prev[0] = insts[-1]

                # two-chunk-lag pipeline state, continuous across pairs/qbs
                pend = []              # [(emit_fn, items)] for previous chunks
                fin_pieces = []        # deferred finalize piece queue
                after_registry = {}    # (qb, pair) -> enqueue-finalize callback

                def pump_fin():
                    if fin_pieces:
                        fin_pieces.pop(0)()

                def flush_one():
                    if pend:
                        emit_fn, items = pend.pop(0)
                        emit_fn(items)

                def flush_pend():
                    while pend:
                        flush_one()

                for qb in range(QB):
                    qsl = slice(qb * 512, (qb + 1) * 512)
                    nkb = 4 * qb + 4
                    # Denominator tiles.  Rows are 32-aligned (partition
                    # shifts from PSUM row 64 must be multiples of 32);
                    # garbage lanes are fine, the reciprocal output is only
                    # read at the real rows.  For qb<3 one a/b tile pair per
                    # q-block (2 reciprocals); for the last q-block one tile
                    # PER HEAD PAIR so each pair's normalization runs as soon
                    # as that pair finishes, keeping the epilogue off the
                    # critical tail.
                    if qb < 3:
                        sums_ab = (
                            s2p.tile([128, 512], f32, tag="sums", bufs=3, name=f"sums_a{qb}"),
                            s2p.tile([128, 512], f32, tag="sums", bufs=3, name=f"sums_b{qb}"),
                            s2p.tile([128, 512], bf16, tag="srec", bufs=3, name=f"srec_a{qb}"),
                            s2p.tile([128, 512], bf16, tag="srec", bufs=3, name=f"srec_b{qb}"),
                        )
                    for pair in range(4):
                        heads = (2 * pair, 2 * pair + 1)
                        if qb < 3:
                            sums_p = None
                        else:
                            sums_p = s2p.tile(
                                [128, 512], f32, tag="sums", bufs=3, name=f"sums{qb}p{pair}"
                            )
                        ovs = {}

                        def make_emit_avs(heads=heads, ovs=ovs, nkb=nkb,
                                          sums_p=sums_p, qsl=qsl, qb=qb,
                                          pair=pair,
                                          sums_ab=(sums_ab if qb < 3 else None),
                                          last=False):
                            def emit_avs(items):
                                if not ovs:
                                    for h in heads:
                                        ovs[h] = ps2.tile(
                                            [DK + 1, 512], f32, tag="ovpo", bufs=2,
                                            name=f"ov{h}",
                                        )
                                grp = []
                                for h in heads:
                                    h2 = h % 2
                                    for (pkb, pcs, pet) in items:
                                        grp.append(nc.tensor.matmul(
                                            ovs[h][:, pcs:], V[:, pkb, h, :],
                                            pet[:, h2, pcs:],
                                            start=(pkb == 0), stop=(pkb == nkb - 1),
                                        ))
                                pe_group(grp)
                                if last:
                                    for h in heads:
                                        ov = ovs[h]
                                        if sums_p is None:
                                            dst = sums_ab[h % 2]
                                            r32 = pair * 32
                                        else:
                                            dst = sums_p
                                            r32 = (h % 2) * 32
                                        nc.vector.tensor_copy(
                                            dst[r32 : r32 + 1, :],
                                            ov[DK : DK + 1, :],
                                        )
                                    for h in heads:
                                        base = 64 * (h % 2)
                                        nc.vector.tensor_copy(
                                            AT[base : base + 64, h // 2, qsl],
                                            ovs[h][0:DK, :],
                                        )
                                    cb = after_registry.pop((qb, pair), None)
                                    if cb is not None:
                                        cb()
                            return emit_avs

                        # balanced chunks of <=3
                        nch = -(-nkb // 3)
                        lo = nkb // nch
                        hi_cnt = nkb - lo * nch
                        sizes = [lo + 1] * hi_cnt + [lo] * (nch - hi_cnt)
                        kbs = list(range(nkb))
                        chunks, pos = [], 0
                        for sz in sizes:
                            chunks.append(kbs[pos : pos + sz])
                            pos += sz
                        for ci, chunk in enumerate(chunks):
                            items = []
                            sc_grp = []
                            for kb in chunk:
                                ksl = slice(kb * 128, (kb + 1) * 128)
                                d = kb - 4 * qb
                                cs = 128 * d if d > 0 else 0
                                sp = ps2.tile(
                                    [128, 2, 512], f32, tag="score", bufs=3,
                                    name=f"sp{pair}q{qb}k{kb}",
                                )
                                for h in heads:
                                    h2 = h % 2
                                    base = 64 * h2
                                    psl = slice(base, base + 64)
                                    sc_grp.append(nc.tensor.matmul(
                                        sp[:, h2, cs:], KT[psl, pair, ksl],
                                        QT[psl, pair, qb * 512 + cs : (qb + 1) * 512],
                                        start=True, stop=True,
                                    ))
                                et = s2p.tile(
                                    [128, 2, 512], bf16, tag="et", bufs=12,
                                    name=f"et{pair}q{qb}k{kb}",
                                )
                                nc.scalar.activation(
                                    et[:, :, cs:], sp[:, :, cs:], AF.Exp, scale=0.125
                                )
                                if d >= 0:
                                    # causal stripe: zero the disallowed
                                    # upper triangle on idle GpSimd
                                    nc.gpsimd.tensor_tensor(
                                        et[:, :, cs : cs + 128],
                                        et[:, :, cs : cs + 128],
                                        maskb[:, :, :],
                                        MULT,
                                    )
                                items.append((kb, cs, et))
                            pe_group(sc_grp)
                            if len(pend) >= 2:
                                flush_one()
                            pump_fin()
                            pend.append(
                                (make_emit_avs(last=(ci == len(chunks) - 1)), items)
                            )

                        if qb < 3:
                            continue
                        srec_p = s2p.tile(
                            [128, 512], bf16, tag="srec", bufs=3, name=f"srec{qb}p{pair}"
                        )

                        def make_recip(sums=sums_p, srec=srec_p):
                            def recip():
                                with nc.allow_low_precision(
                                    reason="denominators are O(1e2); bf16 is plenty"
                                ):
                                    nc.vector.reciprocal(srec[:], sums[:])
                            return recip

                        def make_rb_norm(pair2=pair, qb=qb, qsl=qsl, srec=srec_p):
                            # K=1 broadcast matmul + in-place AT multiply for
                            # one head pair; the rb tiles live only within
                            # this piece.
                            def rb_norm():
                                heads2 = (2 * pair2, 2 * pair2 + 1)
                                rbs = {}
                                rb_grp = []
                                for h in heads2:
                                    r32 = (h % 2) * 32
                                    rb = ps2.tile(
                                        [128, 2, 512], f32, tag="score", bufs=3,
                                        name=f"rb{h}{qb}",
                                    )
                                    rb_grp.append(nc.tensor.matmul(
                                        rb[0:64, 0, :],
                                        ones_r[r32 : r32 + 1, :],
                                        srec[r32 : r32 + 1, :],
                                        start=True, stop=True,
                                    ))
                                    rbs[h] = rb
                                pe_group(rb_grp)
                                for h in heads2:
                                    base = 64 * (h % 2)
                                    nc.vector.tensor_tensor(
                                        AT[base : base + 64, pair2, qsl],
                                        AT[base : base + 64, pair2, qsl],
                                        rbs[h][0:64, 0, :],
                                        MULT,
                                    )
                            return rb_norm

                        pair_fin = [make_recip(), (lambda: None), make_rb_norm()]
                        after_registry[(qb, pair)] = (
                            lambda ps_=pair_fin: fin_pieces.extend(ps_)
                        )
                        if pair == 3:
                            last_pair_fin = pair_fin

                    def make_po_half(sc, jh, og_holder, qb=qb):
                        def po_half():
                            if og_holder[0] is None:
                                og_holder[0] = s2p.tile(
                                    [128, D], bf16, tag="og", bufs=3, name=f"og{sc}"
                                )
                            og = og_holder[0]
                            po = ps2.tile(
                                [128, 2, 512], f32, tag="score", bufs=3,
                                name=f"po{sc}{jh}",
                            )
                            po_grp = []
                            for io in range(4):
                                po_grp.append(nc.tensor.matmul(
                                    po[:, 0, :],
                                    AT[:, io, sc * 128 : (sc + 1) * 128],
                                    wo[:, io, jh * 512 : (jh + 1) * 512],
                                    start=(io == 0), stop=(io == 3),
                                ))
                            pe_group(po_grp)
                            nc.vector.tensor_copy(
                                og[:, jh * 512 : (jh + 1) * 512], po[:, 0, :]
                            )
                            if jh == 1:
                                nc.sync.dma_start(
                                    out_d[sc * 128 : (sc + 1) * 128, :], og[:]
                                )
                        return po_half

                    if qb < 3:
                        qb_fin = []

                        def make_recip_ab(idx, sums_ab=sums_ab):
                            def recip():
                                with nc.allow_low_precision(
                                    reason="denominators are O(1e2); bf16 is plenty"
                                ):
                                    nc.vector.reciprocal(
                                        sums_ab[2 + idx][:], sums_ab[idx][:]
                                    )
                            return recip

                        def make_rb_norm_ab(pair2, qb=qb, qsl=qsl, sums_ab=sums_ab):
                            def rb_norm():
                                heads2 = (2 * pair2, 2 * pair2 + 1)
                                p32 = pair2 * 32
                                rbs = {}
                                rb_grp = []
                                for h in heads2:
                                    srec = sums_ab[2 + (h % 2)]
                                    rb = ps2.tile(
                                        [128, 2, 512], f32, tag="score", bufs=3,
                                        name=f"rb{h}{qb}",
                                    )
                                    rb_grp.append(nc.tensor.matmul(
                                        rb[0:64, 0, :],
                                        ones_r[p32 : p32 + 1, :],
                                        srec[p32 : p32 + 1, :],
                                        start=True, stop=True,
                                        tile_position=(p32, 0) if p32 == 96 else None,
                                    ))
                                    rbs[h] = rb
                                pe_group(rb_grp)
                                for h in heads2:
                                    base = 64 * (h % 2)
                                    nc.vector.tensor_tensor(
                                        AT[base : base + 64, pair2, qsl],
                                        AT[base : base + 64, pair2, qsl],
                                        rbs[h][0:64, 0, :],
                                        MULT,
                                    )
                            return rb_norm

                        qb_fin.append(make_recip_ab(0))
                        qb_fin.append(make_recip_ab(1))
                        qb_fin.append(lambda: None)
                        qb_fin.append(lambda: None)
                        qb_fin.append(lambda: None)
                        for pair2 in range(4):
                            qb_fin.append(make_rb_norm_ab(pair2))
                    else:
                        # for the last q-block the po pieces ride on pair 3's
                        # callback list (mutated before that flush executes)
                        qb_fin = last_pair_fin
                    for sc in range(4 * qb, 4 * qb + 4):
                        og_holder = [None]
                        qb_fin.append(make_po_half(sc, 0, og_holder))
                        qb_fin.append(make_po_half(sc, 1, og_holder))
                    if qb < 3:
                        after_registry[(qb, 3)] = (
                            lambda ps_=qb_fin: fin_pieces.extend(ps_)
                        )

                flush_pend()
                while fin_pieces:
                    pump_fin()

    nc.compile()
    _BUILD_CACHE["nc"] = nc
    return nc


def _host_inputs(x, WQ, WK, WV, WO):
    bf = ml_dtypes.bfloat16
    ki = np.arange(128, dtype=np.float32)[:, None]
    qj = np.arange(128, dtype=np.float32)[None, :]
    # 0/1 stripe mask (multiplied into et after exp): within the partial
    # 128-col stripe of diagonal chunk d, allowed iff local column >= ki.
    # Duplicated side-by-side so one op covers both heads of a pair.
    mask1 = (qj >= ki).astype(np.float32)
    mask = np.concatenate([mask1, mask1], axis=1).astype(bf)
    onesb = np.ones((128, 128), dtype=bf)
    ones32 = np.ones((128, 64), dtype=bf)

    in_maps = []
    for b in range(B):
        xT = np.ascontiguousarray(x[b].T).astype(bf)
        for g in range(G):
            sl = slice(g * DG, (g + 1) * DG)
            in_maps.append(
                {
                    "xT": xT,
                    "wqT": np.ascontiguousarray(WQ[sl, :].T).astype(bf),
                    "wkT": np.ascontiguousarray(WK[sl, :].T).astype(bf),
                    "wvT": np.ascontiguousarray(WV[sl, :].T).astype(bf),
                    "woT": np.ascontiguousarray(WO[:, sl].T).astype(bf),
                    "mask": mask,
                    "onesb": onesb,
                    "ones32": ones32,
                }
            )
    return in_maps


def kernel(x, WQ, WK, WV, WO):
    from concourse.bass_utils import run_bass_kernel_spmd

    x = np.asarray(x, dtype=np.float32)
    WQ = np.asarray(WQ, dtype=np.float32)
    WK = np.asarray(WK, dtype=np.float32)
    WV = np.asarray(WV, dtype=np.float32)
    WO = np.asarray(WO, dtype=np.float32)

    nc = _build()
    in_maps = _host_inputs(x, WQ, WK, WV, WO)
    res = run_bass_kernel_spmd(
        nc,
        in_maps,
        core_ids=list(range(8)),
        trace=bool(os.environ.get("KERNEL_TRACE")),
    )
    kernel.last_results = res
    parts = [r["out"].astype(np.float32) for r in res.results]
    out = np.stack([parts[2 * b] + parts[2 * b + 1] for b in range(B)], axis=0)
    return out.astype(np.float32)


# revision 23
# speedup vs baseline: 1.1889x; 1.1889x over previous
"""Causal multi-head self-attention on 8 Trainium2 NeuronCores.

Sharding: core = (batch b, head-group g).  B=4 batches x 2 groups of 8 heads
= 8 cores.  Each core computes Q/K/V projections for its 8 heads, causal
attention, and a partial output projection (row-shard of WO); the host sums
the two partials per batch (the tensor-parallel all-reduce, done at gather).

v6 (bf16): the attention phase is paced by ScalarE exp (~312 cyc fixed
overhead per ACTIVATE + 1 elem/cycle/lane), so everything else is arranged
to keep the exp stream dense:
  - score tiles for a head PAIR live in one 2-bank PSUM tile [128, 2, 512];
    each k-chunk gets ONE exp covering both heads;
  - the causal stripe mask is a 0/1 multiply on the et tile by the
    otherwise-idle GpSimd engine, off both DVE and the scores->exp path;
  - diagonal-chunk scores/exps are trimmed to the allowed columns [cs:];
  - the scores/attnV software pipeline (attnV lags scores by one chunk) runs
    CONTINUOUSLY across pair and q-block boundaries;
  - the previous q-block's normalization + output projection is sliced into
    small pieces pumped one-per-chunk between attnV groups, filling the PE
    slack under the exp pacing instead of stalling ACT;
  - each 512-wide half of the output projection is a separate 4-matmul
    same-bank accumulation chain (interleaving the halves costs a PE
    pipeline drain per matmul: 760 vs 455 ns measured);
  - ScalarE does nothing but exp in the attention phase.

Stage 1 V runs first per s-block so compute starts as soon as wv + the first
xT chunk land.
"""

import os
import numpy as np
import ml_dtypes

B, S, D = 4, 2048, 1024
H_TOTAL, DK = 16, 64
G = 2          # head groups (cores per batch)
HG = 8         # heads per core
DG = 512       # head dims per core
CO = 8         # contraction chunks of 128 over D
SBLK = 4       # 512-wide s blocks
QB = 4         # 512-wide q blocks
NEG = -1e9

_BUILD_CACHE = {}


def _build():
    if "nc" in _BUILD_CACHE:
        return _BUILD_CACHE["nc"]

    import concourse.bacc as bacc
    import concourse.mybir as mybir
    import concourse.tile as tile
    from concourse.tile_rust import add_dep_helper

    f32 = mybir.dt.float32
    f32r = mybir.dt.float32r
    bf16 = mybir.dt.bfloat16
    AF = mybir.ActivationFunctionType
    MULT = mybir.AluOpType.mult

    nc = bacc.Bacc("TRN2", target_bir_lowering=False)
    xT_d = nc.dram_tensor("xT", [D, S], bf16, kind="ExternalInput")
    wq_d = nc.dram_tensor("wqT", [D, DG], bf16, kind="ExternalInput")
    wk_d = nc.dram_tensor("wkT", [D, DG], bf16, kind="ExternalInput")
    wv_d = nc.dram_tensor("wvT", [D, DG], bf16, kind="ExternalInput")
    wo_d = nc.dram_tensor("woT", [DG, D], bf16, kind="ExternalInput")
    mask_d = nc.dram_tensor("mask", [128, 256], bf16, kind="ExternalInput")
    ones_d = nc.dram_tensor("onesb", [128, 128], bf16, kind="ExternalInput")
    ones32b_d = nc.dram_tensor("ones32", [128, 64], bf16, kind="ExternalInput")
    out_d = nc.dram_tensor("out", [S, D], bf16, kind="ExternalOutput")

    with tile.TileContext(nc) as tc:
        with tc.tile_pool(name="persist", bufs=1) as pp:
            QT = pp.tile([128, 4, S], bf16, tag="QT")
            KT = pp.tile([128, 4, S], bf16, tag="KT")
            V = pp.tile([128, 16, HG, DK + 1], bf16, tag="V")
            AT = pp.tile([128, 4, S], bf16, tag="AT")
            wo = pp.tile([128, 4, D], bf16, tag="wo")
            maskb = pp.tile([128, 2, 128], bf16, tag="maskb")
            onesb = pp.tile([128, 128], bf16, tag="onesb")
            ones_r = pp.tile([128, 64], bf16, tag="ones_r")
            nc.sync.dma_start(maskb[:], mask_d[:, :].rearrange("p (h c) -> p h c", h=2))
            nc.sync.dma_start(onesb[:], ones_d[:, :])
            nc.sync.dma_start(ones_r[:], ones32b_d[:, :])
            # ones column of V (bf16 broadcast copy)
            nc.vector.tensor_copy(
                V[:, :, :, DK : DK + 1],
                onesb[:, 0:128].rearrange("p (so h) -> p so h", so=16)[:, :, :, None],
            )

            # ---------------- stage 1: projections ----------------
            with (
                tc.tile_pool(name="stage1", bufs=1) as s1p,
                tc.tile_pool(name="ps1", bufs=1, space="PSUM") as ps1,
            ):
                wv = s1p.tile([128, CO, DG], bf16, tag="wv")
                wq = s1p.tile([128, CO, DG], bf16, tag="wq")
                wk = s1p.tile([128, CO, DG], bf16, tag="wk")
                nc.sync.dma_start(
                    wv[:, 0:4, :],
                    wv_d[0:512, :].rearrange("(co ci) d -> ci co d", ci=128),
                )
                nc.sync.dma_start(
                    wv[:, 4:8, :],
                    wv_d[512:1024, :].rearrange("(co ci) d -> ci co d", ci=128),
                )
                first = True
                for sb in range(SBLK):
                    xt = s1p.tile([128, CO, 512], bf16, tag="xt", bufs=2)
                    ssl = slice(sb * 512, (sb + 1) * 512)
                    if first:
                        nc.sync.dma_start(
                            xt[:, 0:4, :],
                            xT_d[0:512, ssl].rearrange("(co ci) s -> ci co s", ci=128),
                        )
                        nc.sync.dma_start(
                            xt[:, 4:8, :],
                            xT_d[512:1024, ssl].rearrange("(co ci) s -> ci co s", ci=128),
                        )
                    else:
                        nc.sync.dma_start(
                            xt, xT_d[:, ssl].rearrange("(co ci) s -> ci co s", ci=128)
                        )
                    if first:
                        # queue the remaining weight loads behind the first
                        # x chunk so V(sb0) can start after ~2MB of DMA
                        nc.sync.dma_start(
                            wq, wq_d[:, :].rearrange("(co ci) d -> ci co d", ci=128)
                        )
                        nc.sync.dma_start(
                            wk, wk_d[:, :].rearrange("(co ci) d -> ci co d", ci=128)
                        )
                        nc.sync.dma_start(
                            wo, wo_d[:, :].rearrange("(io ip) j -> ip io j", ip=128)
                        )
                        first = False
                    for so in range(4):
                        sc = sb * 4 + so
                        pv = ps1.tile([128, 512], f32, tag="proj", bufs=4, name=f"pv{sc}")
                        for co in range(CO):
                            nc.tensor.matmul(
                                pv, xt[:, co, so * 128 : (so + 1) * 128], wv[:, co, :],
                                start=(co == 0), stop=(co == CO - 1),
                            )
                        nc.vector.tensor_copy(
                            V[:, sc, :, 0:DK],
                            pv[:].rearrange("p (h d) -> p h d", h=HG),
                        )
                    for do in range(4):
                        dsl = slice(do * 128, (do + 1) * 128)
                        pq = ps1.tile([128, 512], f32, tag="proj", bufs=4, name=f"pq{sb}{do}")
                        for co in range(CO):
                            nc.tensor.matmul(
                                pq, wq[:, co, dsl], xt[:, co, :],
                                start=(co == 0), stop=(co == CO - 1),
                            )
                        nc.vector.tensor_copy(QT[:, do, ssl], pq[:])
                        pk = ps1.tile([128, 512], f32, tag="proj", bufs=4, name=f"pk{sb}{do}")
                        for co in range(CO):
                            nc.tensor.matmul(
                                pk, wk[:, co, dsl], xt[:, co, :],
                                start=(co == 0), stop=(co == CO - 1),
                            )
                        nc.vector.tensor_copy(KT[:, do, ssl], pk[:])

            # ---------------- stages 2+3: attention + output ----------------
            with (
                tc.tile_pool(name="stage2", bufs=1) as s2p,
                tc.tile_pool(name="ps2", bufs=1, space="PSUM") as ps2,
            ):
                pe_prev = [None]  # last instr of the previous PE group

                def pe_group(insts):
                    if not insts:
                        return
                    if pe_prev[0] is not None:
                        add_dep_helper(
                            insts[0].ins, pe_prev[0].ins, sync=False,
                            reason="pe group order",
                        )
                    for a, b in zip(insts[1:], insts):
                        add_dep_helper(a.ins, b.ins, sync=False, reason="pe chain")
                    pe_prev[0] = insts[-1]

                # two-chunk-lag pipeline state, continuous across pairs/qbs
                pend = []              # [(emit_fn, items)] for previous chunks
                fin_pieces = []        # deferred finalize piece queue
                after_registry = {}    # (qb, pair) -> enqueue-finalize callback

                def pump_fin():
                    if fin_pieces:
                        fin_pieces.pop(0)()

                def flush_one():
                    if pend:
                        emit_fn, items = pend.pop(0)
                        emit_fn(items)

                def flush_pend():
                    while pend:
                        flush_one()

                for qb in range(QB):
                    qsl = slice(qb * 512, (qb + 1) * 512)
                    nkb = 4 * qb + 4
                    # Denominator tiles.  Rows are 32-aligned (partition
                    # shifts from PSUM row 64 must be multiples of 32);
                    # garbage lanes are fine, the reciprocal output is only
                    # read at the real rows.  For qb<3 one a/b tile pair per
                    # q-block (2 reciprocals); for the last q-block one tile
                    # PER HEAD PAIR so each pair's normalization runs as soon
                    # as that pair finishes, keeping the epilogue off the
                    # critical tail.
                    if qb < 3:
                        sums_ab = (
                            s2p.tile([128, 512], f32, tag="sums", bufs=3, name=f"sums_a{qb}"),
                            s2p.tile([128, 512], f32, tag="sums", bufs=3, name=f"sums_b{qb}"),
                            s2p.tile([128, 512], bf16, tag="srec", bufs=3, name=f"srec_a{qb}"),
                            s2p.tile([128, 512], bf16, tag="srec", bufs=3, name=f"srec_b{qb}"),
                        )
                    for pair in range(4):
                        heads = (2 * pair, 2 * pair + 1)
                        if qb < 3:
                            sums_p = None
                        else:
                            sums_p = s2p.tile(
                                [128, 512], f32, tag="sums", bufs=3, name=f"sums{qb}p{pair}"
                            )
                        ovs = {}

                        def make_emit_avs(heads=heads, ovs=ovs, nkb=nkb,
                                          sums_p=sums_p, qsl=qsl, qb=qb,
                                          pair=pair,
                                          sums_ab=(sums_ab if qb < 3 else None),
                                          last=False):
                            def emit_avs(items):
                                if not ovs:
                                    for h in heads:
                                        ovs[h] = ps2.tile(
                                            [DK + 1, 512], f32, tag="ovpo", bufs=2,
                                            name=f"ov{h}",
                                        )
                                grp = []
                                for h in heads:
                                    h2 = h % 2
                                    for (pkb, pcs, pet) in items:
                                        grp.append(nc.tensor.matmul(
                                            ovs[h][:, pcs:], V[:, pkb, h, :],
                                            pet[:, h2, pcs:],
                                            start=(pkb == 0), stop=(pkb == nkb - 1),
                                        ))
                                pe_group(grp)
                                if last:
                                    for h in heads:
                                        ov = ovs[h]
                                        if sums_p is None:
                                            dst = sums_ab[h % 2]
                                            r32 = pair * 32
                                        else:
                                            dst = sums_p
                                            r32 = (h % 2) * 32
                                        nc.vector.tensor_copy(
                                            dst[r32 : r32 + 1, :],
                                            ov[DK : DK + 1, :],
                                        )
                                    for h in heads:
                                        base = 64 * (h % 2)
                                        nc.vector.tensor_copy(
                                            AT[base : base + 64, h // 2, qsl],
                                            ovs[h][0:DK, :],
                                        )
                                    cb = after_registry.pop((qb, pair), None)
                                    if cb is not None:
                                        cb()
                            return emit_avs

                        # balanced chunks of <=3
                        nch = -(-nkb // 3)
                        lo = nkb // nch
                        hi_cnt = nkb - lo * nch
                        sizes = [lo + 1] * hi_cnt + [lo] * (nch - hi_cnt)
                        kbs = list(range(nkb))
                        chunks, pos = [], 0
                        for sz in sizes:
                            chunks.append(kbs[pos : pos + sz])
                            pos += sz
                        for ci, chunk in enumerate(chunks):
                            items = []
                            sc_grp = []
                            for kb in chunk:
                                ksl = slice(kb * 128, (kb + 1) * 128)
                                d = kb - 4 * qb
                                cs = 128 * d if d > 0 else 0
                                sp = ps2.tile(
                                    [128, 2, 512], f32, tag="score", bufs=3,
                                    name=f"sp{pair}q{qb}k{kb}",
                                )
                                for h in heads:
                                    h2 = h % 2
                                    base = 64 * h2
                                    psl = slice(base, base + 64)
                                    sc_grp.append(nc.tensor.matmul(
                                        sp[:, h2, cs:], KT[psl, pair, ksl],
                                        QT[psl, pair, qb * 512 + cs : (qb + 1) * 512],
                                        start=True, stop=True,
                                    ))
                                et = s2p.tile(
                                    [128, 2, 512], bf16, tag="et", bufs=16,
                                    name=f"et{pair}q{qb}k{kb}",
                                )
                                nc.scalar.activation(
                                    et[:, :, cs:], sp[:, :, cs:], AF.Exp, scale=0.125
                                )
                                if d >= 0:
                                    # causal stripe: zero the disallowed
                                    # upper triangle on idle GpSimd
                                    nc.gpsimd.tensor_tensor(
                                        et[:, :, cs : cs + 128],
                                        et[:, :, cs : cs + 128],
                                        maskb[:, :, :],
                                        MULT,
                                    )
                                items.append((kb, cs, et))
                            pe_group(sc_grp)
                            if len(pend) >= 2:
                                flush_one()
                            pump_fin()
                            pend.append(
                                (make_emit_avs(last=(ci == len(chunks) - 1)), items)
                            )

                        if qb < 3:
                            continue
                        srec_p = s2p.tile(
                            [128, 512], bf16, tag="srec", bufs=3, name=f"srec{qb}p{pair}"
                        )

                        def make_recip(half, sums=sums_p, srec=srec_p):
                            def recip():
                                h0, h1 = half * 256, (half + 1) * 256
                                with nc.allow_low_precision(
                                    reason="denominators are O(1e2); bf16 is plenty"
                                ):
                                    nc.vector.reciprocal(
                                        srec[:, h0:h1], sums[:, h0:h1]
                                    )
                            return recip

                        def make_rb_norm(pair2=pair, qb=qb, qsl=qsl, srec=srec_p):
                            # K=1 broadcast matmul + in-place AT multiply for
                            # one head pair; the rb tiles live only within
                            # this piece.
                            def rb_norm():
                                heads2 = (2 * pair2, 2 * pair2 + 1)
                                rbs = {}
                                rb_grp = []
                                for h in heads2:
                                    r32 = (h % 2) * 32
                                    rb = ps2.tile(
                                        [128, 2, 512], f32, tag="score", bufs=3,
                                        name=f"rb{h}{qb}",
                                    )
                                    rb_grp.append(nc.tensor.matmul(
                                        rb[0:64, 0, :],
                                        ones_r[r32 : r32 + 1, :],
                                        srec[r32 : r32 + 1, :],
                                        start=True, stop=True,
                                    ))
                                    rbs[h] = rb
                                pe_group(rb_grp)
                                for h in heads2:
                                    base = 64 * (h % 2)
                                    nc.vector.tensor_tensor(
                                        AT[base : base + 64, pair2, qsl],
                                        AT[base : base + 64, pair2, qsl],
                                        rbs[h][0:64, 0, :],
                                        MULT,
                                    )
                            return rb_norm

                        pair_fin = [make_recip(0), make_recip(1),
                                    (lambda: None), make_rb_norm()]
                        after_registry[(qb, pair)] = (
                            lambda ps_=pair_fin: fin_pieces.extend(ps_)
                        )
                        if pair == 3:
                            last_pair_fin = pair_fin

                    def make_po_half(sc, jh, og_holder, qb=qb):
                        def po_half():
                            if og_holder[0] is None:
                                og_holder[0] = s2p.tile(
                                    [128, D], bf16, tag="og", bufs=3, name=f"og{sc}"
                                )
                            og = og_holder[0]
                            po = ps2.tile(
                                [128, 2, 512], f32, tag="score", bufs=3,
                                name=f"po{sc}{jh}",
                            )
                            po_grp = []
                            for io in range(4):
                                po_grp.append(nc.tensor.matmul(
                                    po[:, 0, :],
                                    AT[:, io, sc * 128 : (sc + 1) * 128],
                                    wo[:, io, jh * 512 : (jh + 1) * 512],
                                    start=(io == 0), stop=(io == 3),
                                ))
                            pe_group(po_grp)
                            nc.vector.tensor_copy(
                                og[:, jh * 512 : (jh + 1) * 512], po[:, 0, :]
                            )
                            if jh == 1:
                                nc.sync.dma_start(
                                    out_d[sc * 128 : (sc + 1) * 128, :], og[:]
                                )
                        return po_half

                    if qb < 3:
                        qb_fin = []

                        def make_recip_ab(idx, half, sums_ab=sums_ab):
                            def recip():
                                h0, h1 = half * 256, (half + 1) * 256
                                with nc.allow_low_precision(
                                    reason="denominators are O(1e2); bf16 is plenty"
                                ):
                                    nc.vector.reciprocal(
                                        sums_ab[2 + idx][:, h0:h1],
                                        sums_ab[idx][:, h0:h1],
                                    )
                            return recip

                        def make_rb_norm_ab(pair2, qb=qb, qsl=qsl, sums_ab=sums_ab):
                            def rb_norm():
                                heads2 = (2 * pair2, 2 * pair2 + 1)
                                p32 = pair2 * 32
                                rbs = {}
                                rb_grp = []
                                for h in heads2:
                                    srec = sums_ab[2 + (h % 2)]
                                    rb = ps2.tile(
                                        [128, 2, 512], f32, tag="score", bufs=3,
                                        name=f"rb{h}{qb}",
                                    )
                                    rb_grp.append(nc.tensor.matmul(
                                        rb[0:64, 0, :],
                                        ones_r[p32 : p32 + 1, :],
                                        srec[p32 : p32 + 1, :],
                                        start=True, stop=True,
                                        tile_position=(p32, 0) if p32 == 96 else None,
                                    ))
                                    rbs[h] = rb
                                pe_group(rb_grp)
                                for h in heads2:
                                    base = 64 * (h % 2)
                                    nc.vector.tensor_tensor(
                                        AT[base : base + 64, pair2, qsl],
                                        AT[base : base + 64, pair2, qsl],
                                        rbs[h][0:64, 0, :],
                                        MULT,
                                    )
                            return rb_norm

                        qb_fin.append(make_recip_ab(0, 0))
                        qb_fin.append(make_recip_ab(0, 1))
                        qb_fin.append(make_recip_ab(1, 0))
                        qb_fin.append(make_recip_ab(1, 1))
                        qb_fin.append(lambda: None)
                        for pair2 in range(4):
                            qb_fin.append(make_rb_norm_ab(pair2))
                    else:
                        # for the last q-block the po pieces ride on pair 3's
                        # callback list (mutated before that flush executes)
                        qb_fin = last_pair_fin
                    for sc in range(4 * qb, 4 * qb + 4):
                        og_holder = [None]
                        qb_fin.append(make_po_half(sc, 0, og_holder))
                        qb_fin.append(make_po_half(sc, 1, og_holder))
                    if qb < 3:
                        after_registry[(qb, 3)] = (
                            lambda ps_=qb_fin: fin_pieces.extend(ps_)
                        )

                flush_pend()
                while fin_pieces:
                    pump_fin()

    nc.compile()
    _BUILD_CACHE["nc"] = nc
    return nc


def _host_inputs(x, WQ, WK, WV, WO):
    bf = ml_dtypes.bfloat16
    ki = np.arange(128, dtype=np.float32)[:, None]
    qj = np.arange(128, dtype=np.float32)[None, :]
    # 0/1 stripe mask (multiplied into et after exp): within the partial
    # 128-col stripe of diagonal chunk d, allowed iff local column >= ki.
    # Duplicated side-by-side so one op covers both heads of a pair.
    mask1 = (qj >= ki).astype(np.float32)
    mask = np.concatenate([mask1, mask1], axis=1).astype(bf)
    onesb = np.ones((128, 128), dtype=bf)
    ones32 = np.ones((128, 64), dtype=bf)

    in_maps = []
    for b in range(B):
        xT = np.ascontiguousarray(x[b].T).astype(bf)
        for g in range(G):
            sl = slice(g * DG, (g + 1) * DG)
            in_maps.append(
                {
                    "xT": xT,
                    "wqT": np.ascontiguousarray(WQ[sl, :].T).astype(bf),
                    "wkT": np.ascontiguousarray(WK[sl, :].T).astype(bf),
                    "wvT": np.ascontiguousarray(WV[sl, :].T).astype(bf),
                    "woT": np.ascontiguousarray(WO[:, sl].T).astype(bf),
                    "mask": mask,
                    "onesb": onesb,
                    "ones32": ones32,
                }
            )
    return in_maps


def kernel(x, WQ, WK, WV, WO):
    from concourse.bass_utils import run_bass_kernel_spmd

    x = np.asarray(x, dtype=np.float32)
    WQ = np.asarray(WQ, dtype=np.float32)
    WK = np.asarray(WK, dtype=np.float32)
    WV = np.asarray(WV, dtype=np.float32)
    WO = np.asarray(WO, dtype=np.float32)

    nc = _build()
    in_maps = _host_inputs(x, WQ, WK, WV, WO)
    res = run_bass_kernel_spmd(
        nc,
        in_maps,
        core_ids=list(range(8)),
        trace=bool(os.environ.get("KERNEL_TRACE")),
    )
    kernel.last_results = res
    parts = [r["out"].astype(np.float32) for r in res.results]
    out = np.stack([parts[2 * b] + parts[2 * b + 1] for b in range(B)], axis=0)
    return out.astype(np.float32)
